# revision 36
# baseline (speedup 1.0000x reference)
"""InteractionNet GNN message-passing kernel for 8 TRN2 NeuronCores.

Data-parallel over batch B=8: core b handles batch element b entirely
locally (no collectives). Weights are replicated to every core.

Per-core math (x1 [256,128], x2 [256,128], ve [256,256]):
  Mx2[j,g] = x2 @ M_w.T + M_b
  m2[i,g]  = max_j(Mx2[j,g] * ve[i,j])         (ve is 0/1)
  x        = relu(m1 + m2), m1 = x1 @ W_w.T + W_b
  GRU(x, x1) -> out

The masked max is computed with a log-sum-exp relaxation on the PE:
  E[j,g]  = exp(t*Mx2nb[j,g] - t*colmax[g])   (t folded into M_w on
            host; colmax via DVE reduce -- the HW exp table is only
            accurate near 0 so the max term must sit at exactly 0)
  S[i,g]  = sum_j ve[i,j] * E[j,g]            (PE matmul)
  m2[i,g] = max(0, ln(S)/t + colmax + M_b[g])
With t=32 the softening error measures ~3.8e-3 end-to-end (gate 2e-2).
The zero floor (masked entries) is exact via the final Relu.

Perf notes:
- DMAs are descriptor-bound (~12ns per partition row): everything bf16
  rides in two fat packs, each split into partition halves across the
  two HWDGE queues (sync + scalar) so the 128-row descriptor cost
  halves and both queues stream concurrently.
- Exp + E-transpose are split per j-half so the S matmul starts after
  the first half's transpose lands.
- GRU: one PSUM bank per tile holds [rz_sum | gi_n | gh_n]; the
  x1-side matmuls and the bias row pre-run in idle PE slots before xT
  exists, leaving only the gi matmul on the post-xT critical path.
- ACT tables ({Exp,Ln} vs {Sigmoid,Tanh}) cost ~1.28us per switch: a
  dummy Exp warms the first during the DMA wait, a dummy Sigmoid
  data-pinned on m2T warms the second while DVE finishes xs/xT.
- xT relu runs on DVE (tensor_scalar add-bias + max0) to free ACT.
"""
import numpy as np
import ml_dtypes

import concourse.bass as bass
import concourse.bacc as bacc
import concourse.mybir as mybir
from concourse.tile import TileContext
from concourse.masks import make_identity
from concourse.bass_utils import run_bass_kernel_spmd

B, N1, N2, F = 8, 256, 256, 128
F3 = 3 * F
DT = mybir.dt.float32
BF = mybir.dt.bfloat16
AF = mybir.ActivationFunctionType
ALU = mybir.AluOpType
P = 128
H = 64              # partition half for split DMAs
T = 32.0            # LSE temperature


def build():
    nc = bass.Bass()
    # crit: x2T [.,256] | T*M_wT [.,128]
    crit = nc.declare_dram_parameter("crit", [P, 384], BF, isOutput=False)
    # veTp: veT0 [.,256] | veT1 [.,256]
    veTp = nc.declare_dram_parameter("veTp", [P, 512], BF, isOutput=False)
    # x1bfp: x1 rows 0:128 | x1 rows 128:256  (bf16: x1 only enters
    #        the output linearly via z*x1, ~0.1%% extra error)
    x1bfp = nc.declare_dram_parameter("x1bfp", [P, 256], BF, isOutput=False)
    # pB1: x1T [.,256] | W_wT [.,128]
    pB1 = nc.declare_dram_parameter("pB1", [P, 384], BF, isOutput=False)
    wihTp = nc.declare_dram_parameter("wihTp", [P, 384], BF, isOutput=False)
    whhTp = nc.declare_dram_parameter("whhTp", [P, 384], BF, isOutput=False)
    # bcols: W_b col | M_b col   (fp32 per-partition bias columns)
    bcols = nc.declare_dram_parameter("bcols", [P, 2], DT, isOutput=False)
    # brow: (bih+bhh)[0:2F] | bih[2F:3F] | bhh[2F:3F]   (bf16 row)
    brow = nc.declare_dram_parameter("brow", [1, 512], BF, isOutput=False)
    # bf16 out: h is computed fp32 on-chip, host casts back (~0.2%%)
    out = nc.declare_dram_parameter("out", [P, 2 * F], BF, isOutput=True)

    with TileContext(nc) as tc:
        with (
            tc.tile_pool(name="const", bufs=1) as const,
            tc.tile_pool(name="gp", bufs=4) as gp,
            tc.tile_pool(name="tp", bufs=2, space="PSUM") as tp,
            tc.tile_pool(name="mmp", bufs=1, space="PSUM") as mmp,
            tc.tile_pool(name="grup", bufs=1, space="PSUM") as grup,
        ):
            # ---- tiny setup (no DMA deps) ----
            dum = const.tile([1, 1], DT, tag="dum")
            nc.vector.memset(dum[:], 1.0)
            epsb = const.tile([P, 1], DT, tag="epsb")
            nc.vector.memset(epsb[:], 1e-36)
            dumob = const.tile([1, 1], BF, tag="dumob")
            # warm the ACT exp/ln table while DMAs are in flight
            nc.scalar.activation(dumob[:], dum[:], AF.Exp,
                                 bias=epsb[0:1, :])
            ident_bf = const.tile([P, P], BF, tag="ident_bf")
            make_identity(nc, ident_bf)
            ones_bf = const.tile([1, P], BF, tag="ones_bf")
            nc.vector.memset(ones_bf[:], 1.0)

            # ---- input DMAs: partition-halved across both HWDGE
            #      queues, strict need-order per queue; the tiny bias
            #      tensors ride the gpsimd SWDGE queue so they land
            #      early without stealing HWDGE bandwidth ----
            crit_s = const.tile([P, 384], BF, tag="crit_s")
            veT_s = const.tile([P, 512], BF, tag="veT_s")
            x1bf_s = const.tile([P, 256], BF, tag="x1bf_s")
            pB1_s = const.tile([P, 384], BF, tag="pB1_s")
            wihT_s = const.tile([P, 384], BF, tag="wihT_s")
            whhT_s = const.tile([P, 384], BF, tag="whhT_s")
            bcols_s = const.tile([P, 2], DT, tag="bcols_s")
            brow_s = const.tile([1, 512], BF, tag="brow_s")
            nc.gpsimd.dma_start(out=brow_s[:], in_=brow[:])
            nc.gpsimd.dma_start(out=bcols_s[:], in_=bcols[:])
            # crit and veT ride both ~55GB/s HWDGE queues as partition
            # halves (critical order); the rest is byte-balanced so every
            # matmul operand lands before the PE needs it
            nc.sync.dma_start(out=crit_s[0:H, :], in_=crit[0:H, :])
            nc.scalar.dma_start(out=crit_s[H:P, :], in_=crit[H:P, :])
            nc.sync.dma_start(out=veT_s[0:H, :], in_=veTp[0:H, :])
            nc.scalar.dma_start(out=veT_s[H:P, :], in_=veTp[H:P, :])
            nc.sync.dma_start(out=whhT_s[:], in_=whhTp[:])
            nc.scalar.dma_start(out=pB1_s[:], in_=pB1[:])
            nc.sync.dma_start(out=x1bf_s[:], in_=x1bfp[:])
            nc.scalar.dma_start(out=wihT_s[:], in_=wihTp[:])

            x2T = crit_s[:, 0:256]
            M_wT = crit_s[:, 256:384]     # pre-scaled by T on host
            veT0 = veT_s[:, 0:256]
            veT1 = veT_s[:, 256:512]
            x1T = pB1_s[:, 0:256]
            W_wT = pB1_s[:, 256:384]
            wihT = wihT_s
            whhT = whhT_s
            x1bf = x1bf_s
            wbcol = bcols_s[:, 0:1]
            mbcol = bcols_s[:, 1:2]

            # ---- pmx[g,j] = T * (x2 @ M_w.T).T  (biasless) ----
            pmx = mmp.tile([P, N2], DT, tag="mm256", name="pmx")
            nc.tensor.matmul(pmx[:], lhsT=M_wT, rhs=x2T,
                             start=True, stop=True)
            negt = const.tile([P, 1], DT, tag="negt")
            nc.vector.tensor_reduce(out=negt[:], in_=pmx[:],
                                    axis=mybir.AxisListType.X, op=ALU.max,
                                    negate=True)
            # per j-half: exp -> PE transpose -> copy, pipelined
            ET = const.tile([P, N2], BF, tag="ET")
            E0 = const.tile([P, F], BF, tag="E0")
            E1 = const.tile([P, F], BF, tag="E1")
            for k, Ek in enumerate((E0, E1)):
                ks = slice(k * P, (k + 1) * P)
                nc.scalar.activation(ET[:, ks], pmx[:, ks], AF.Exp,
                                     bias=negt[:])
                pe = tp.tile([P, P], BF, tag="pe")
                nc.tensor.transpose(pe[:], ET[:, ks], ident_bf[:])
                nc.vector.tensor_copy(Ek[:], pe[:])

            # ---- S^T[g,i] = sum_j E[j,g] * veT[j,i]  (PE) ----
            pst = mmp.tile([P, N1], DT, tag="pst")
            nc.tensor.matmul(pst[:], lhsT=E0[:], rhs=veT0,
                             start=True, stop=False)
            nc.tensor.matmul(pst[:], lhsT=E1[:], rhs=veT1,
                             start=False, stop=True)
            # m1T (biasless) = (x1 @ W_w.T).T  -- fills the PE slot
            # between the S halves' dependencies
            pm1 = mmp.tile([P, N1], DT, tag="mm256", name="pm1")
            nc.tensor.matmul(pm1[:], lhsT=W_wT, rhs=x1T,
                             start=True, stop=True)

            # colmax2 = colmax + M_b = -negt/T + M_b  (for the m2 relu)
            colmax2 = const.tile([P, 1], DT, tag="colmax2")
            nc.vector.tensor_scalar(colmax2[:], negt[:], -1.0 / T, mbcol,
                                    op0=ALU.mult, op1=ALU.add)

            # ---- GRU pre-runs (no xT dependency): per tile PSUM bank
            #      PG = [gi_rz+gh_rz+b_rz (0:256) | gi_n+bih_n (256:384)
            #            | gh_n+bhh_n (384:512)] ----
            PGs = []
            for nt in range(2):
                ns = slice(nt * P, (nt + 1) * P)
                PG = grup.tile([P, 4 * F], DT, tag="PG", bufs=2,
                               name=f"PG{nt}")
                # bias row opens the group (zero-init + bias everywhere);
                # brow layout matches [b_rz | bih_n | bhh_n]
                nc.tensor.matmul(PG[:], lhsT=ones_bf[:], rhs=brow_s[:],
                                 start=True, stop=False)
                nc.tensor.matmul(PG[:, 0:2 * F], lhsT=x1T[:, ns],
                                 rhs=whhT[:, 0:2 * F], start=False, stop=False,
                                 skip_group_check=True)
                nc.tensor.matmul(PG[:, 3 * F:4 * F], lhsT=x1T[:, ns],
                                 rhs=whhT[:, 2 * F:3 * F], start=False, stop=False,
                                 skip_group_check=True)
                PGs.append(PG)

            # ---- LSE tail ----
            lnS = gp.tile([P, N1], DT, tag="lnS")
            nc.scalar.activation(lnS[:], pst[:], AF.Ln, bias=epsb[:])
            m2T = gp.tile([P, N1], DT, tag="m2T")
            nc.scalar.activation(m2T[:], lnS[:], AF.Relu,
                                 bias=colmax2[:], scale=1.0 / T)
            # warm the sigmoid/tanh table now -- ACT is done until the
            # GRU; the m2T read pins this after the Relu
            dumo2 = const.tile([1, 1], DT, tag="dumo2")
            nc.scalar.activation(dumo2[:], m2T[0:1, 0:1], AF.Sigmoid)
            # xs/xT on DVE so the table load overlaps them
            xs = gp.tile([P, N1], DT, tag="xs")
            nc.vector.tensor_add(xs[:], pm1[:], m2T[:])
            xT = const.tile([P, N1], BF, tag="xT")
            nc.vector.tensor_scalar(xT[:], xs[:], wbcol, 0.0,
                                    op0=ALU.add, op1=ALU.max)

            # ---- GRU post-xT: gi matmul, then the vector tail ----
            for nt in range(2):
                ns = slice(nt * P, (nt + 1) * P)
                x1_p = x1bf[:, ns]
                PG = PGs[nt]
                beng = nc.vector if nt == 0 else nc.gpsimd
                nc.tensor.matmul(PG[:, 0:F3], lhsT=xT[:, ns], rhs=wihT[:],
                                 start=False, stop=True,
                                 skip_group_check=True)
                rz = gp.tile([P, 2 * F], DT, tag="rz")
                nc.scalar.activation(rz[:], PG[:, 0:2 * F], AF.Sigmoid)
                rr, zz = rz[:, 0:F], rz[:, F:2 * F]
                # gpsimd can't read PSUM: t1/t2 stay on DVE
                t1 = gp.tile([P, F], DT, tag="t1")
                nc.vector.tensor_mul(t1[:], rr, PG[:, 3 * F:4 * F])
                t2 = gp.tile([P, F], DT, tag="t2")
                nc.vector.tensor_add(t2[:], t1[:], PG[:, 2 * F:F3])
                nn = gp.tile([P, F], DT, tag="nn")
                nc.scalar.activation(nn[:], t2[:], AF.Tanh)
                # omz/zx overlap the tanh; only t5/hh follow it
                omz = gp.tile([P, F], DT, tag="omz")
                beng.tensor_scalar(omz[:], zz, -1.0, 1.0,
                                   op0=ALU.mult, op1=ALU.add)
                zx = gp.tile([P, F], DT, tag="zx")
                beng.tensor_mul(zx[:], zz, x1_p)
                t5 = gp.tile([P, F], DT, tag="t5")
                nc.vector.tensor_mul(t5[:], omz[:], nn[:])
                hh = gp.tile([P, F], BF, tag=f"hh{nt}", name=f"hh{nt}")
                nc.vector.tensor_add(hh[:], t5[:], zx[:])
                # split each tile's output across both queues
                nc.scalar.dma_start(out=out[0:H, ns], in_=hh[0:H, :])
                nc.sync.dma_start(out=out[H:P, ns], in_=hh[H:P, :])

    # Walrus's TRN2 codegen allows at most one sync wait per instruction
    # (S3 LW struct). These Bacc passes split/move the extra waits.
    import bass_rust as _bass_rust
    _bass_rust.move_matmul_waits_to_ldweights(nc.m)
    bacc.Bacc.generate_event_semaphores(nc)
    bacc.Bacc.insert_library_loads(nc)
    mybir.codegen_inst_isa_subclasses(nc)
    return nc


_NC = None


def _in_maps(inputs):
    f32 = lambda a: np.ascontiguousarray(np.asarray(a), dtype=np.float32)
    bf = lambda a: np.ascontiguousarray(
        np.asarray(a, dtype=np.float32).astype(ml_dtypes.bfloat16))
    x1, x2, ve = (f32(inputs[k]) for k in ("x1", "x2", "valid_edge"))
    W_w, M_w = f32(inputs["W_w"]), f32(inputs["M_w"])
    W_b, M_b = f32(inputs["W_b"]), f32(inputs["M_b"])
    wih, whh = f32(inputs["gru_wih"]), f32(inputs["gru_whh"])
    bih, bhh = f32(inputs["gru_bih"]), f32(inputs["gru_bhh"])

    brow = np.empty((1, 512), np.float32)
    brow[0, 0:256] = bih[0:256] + bhh[0:256]
    brow[0, 256:384] = bih[256:384]
    brow[0, 384:512] = bhh[256:384]
    brow = bf(brow)

    M_wTb, W_wTb = bf(T * M_w.T), bf(W_w.T)
    wihTb, whhTb = bf(wih.T), bf(whh.T)
    bcols = np.empty((P, 2), np.float32)
    bcols[:, 0] = W_b
    bcols[:, 1] = M_b
    maps = []
    for b in range(B):
        veTb = bf(ve[b].T)
        crit = np.concatenate([bf(x2[b].T), M_wTb], axis=1)
        veTp = np.concatenate([veTb[0:P], veTb[P:2 * P]], axis=1)
        x1b = bf(x1[b])
        x1bfp = np.concatenate([x1b[0:P], x1b[P:2 * P]], axis=1)
        pB1 = np.concatenate([bf(x1[b].T), W_wTb], axis=1)
        maps.append({"crit": np.ascontiguousarray(crit),
                     "veTp": np.ascontiguousarray(veTp),
                     "x1bfp": np.ascontiguousarray(x1bfp),
                     "pB1": np.ascontiguousarray(pB1),
                     "wihTp": wihTb, "whhTp": whhTb,
                     "bcols": bcols, "brow": brow})
    return maps


def kernel(**inputs):
    global _NC
    if _NC is None:
        _NC = build()
    res = run_bass_kernel_spmd(_NC, _in_maps(inputs), list(range(B)))
    outs = []
    for b in range(B):
        o = np.asarray(res.results[b]["out"], dtype=np.float32)
        outs.append(np.concatenate([o[:, 0:F], o[:, F:2 * F]], axis=0))
    return np.stack(outs, axis=0).astype(np.float32)


# revision 37
# speedup vs baseline: 1.1454x; 1.1454x over previous
"""InteractionNet GNN message-passing kernel for 8 TRN2 NeuronCores.

Data-parallel over batch B=8: core b handles batch element b entirely
locally (no collectives). Weights are replicated to every core.

Per-core math (x1 [256,128], x2 [256,128], ve [256,256]):
  Mx2[j,g] = x2 @ M_w.T + M_b
  m2[i,g]  = max_j(Mx2[j,g] * ve[i,j])         (ve is 0/1)
  x        = relu(m1 + m2), m1 = x1 @ W_w.T + W_b
  GRU(x, x1) -> out

The masked max is computed with a log-sum-exp relaxation on the PE:
  E[j,g]  = exp(t*Mx2nb[j,g] - t*colmax[g])   (t folded into M_w on
            host; colmax via DVE reduce -- the HW exp table is only
            accurate near 0 so the max term must sit at exactly 0)
  S[i,g]  = sum_j ve[i,j] * E[j,g]            (PE matmul)
  m2[i,g] = max(0, ln(S)/t + colmax + M_b[g])
With t=32 the softening error measures ~3.8e-3 end-to-end (gate 2e-2).
The zero floor (masked entries) is exact via the final Relu.

Perf notes:
- DMAs are descriptor-bound (~12ns per partition row): everything bf16
  rides in two fat packs, each split into partition halves across the
  two HWDGE queues (sync + scalar) so the 128-row descriptor cost
  halves and both queues stream concurrently.
- Exp + E-transpose are split per j-half so the S matmul starts after
  the first half's transpose lands.
- GRU: one PSUM bank per tile holds [rz_sum | gi_n | gh_n]; the
  x1-side matmuls and the bias row pre-run in idle PE slots before xT
  exists, leaving only the gi matmul on the post-xT critical path.
- ACT tables ({Exp,Ln} vs {Sigmoid,Tanh}) cost ~1.28us per switch: a
  dummy Exp warms the first during the DMA wait, a dummy Sigmoid
  data-pinned on m2T warms the second while DVE finishes xs/xT.
- xT relu runs on DVE (tensor_scalar add-bias + max0) to free ACT.
"""
import numpy as np
import ml_dtypes

import concourse.bass as bass
import concourse.bacc as bacc
import concourse.mybir as mybir
from concourse.tile import TileContext
from concourse.masks import make_identity
from concourse.bass_utils import run_bass_kernel_spmd

B, N1, N2, F = 8, 256, 256, 128
F3 = 3 * F
DT = mybir.dt.float32
BF = mybir.dt.bfloat16
AF = mybir.ActivationFunctionType
ALU = mybir.AluOpType
P = 128
H = 64              # partition half for split DMAs
T = 32.0            # LSE temperature


def build():
    nc = bass.Bass()
    # crit: x2T [.,256] | T*M_wT [.,128]
    crit = nc.declare_dram_parameter("crit", [P, 384], BF, isOutput=False)
    # veTp: veT0 [.,256] | veT1 [.,256]
    veTp = nc.declare_dram_parameter("veTp", [P, 512], BF, isOutput=False)
    # x1bfp: x1 rows 0:128 | x1 rows 128:256  (bf16: x1 only enters
    #        the output linearly via z*x1, ~0.1%% extra error)
    x1bfp = nc.declare_dram_parameter("x1bfp", [P, 256], BF, isOutput=False)
    # pB1: x1T [.,256] | W_wT [.,128]
    pB1 = nc.declare_dram_parameter("pB1", [P, 384], BF, isOutput=False)
    wihTp = nc.declare_dram_parameter("wihTp", [P, 384], BF, isOutput=False)
    whhTp = nc.declare_dram_parameter("whhTp", [P, 384], BF, isOutput=False)
    # bcols: W_b col | M_b col   (fp32 per-partition bias columns)
    bcols = nc.declare_dram_parameter("bcols", [P, 2], DT, isOutput=False)
    # brow: (bih+bhh)[0:2F] | bih[2F:3F] | bhh[2F:3F]   (bf16 row)
    brow = nc.declare_dram_parameter("brow", [1, 512], BF, isOutput=False)
    # bf16 out: h is computed fp32 on-chip, host casts back (~0.2%%)
    out = nc.declare_dram_parameter("out", [P, 2 * F], BF, isOutput=True)

    with TileContext(nc) as tc:
        with (
            tc.tile_pool(name="const", bufs=1) as const,
            tc.tile_pool(name="gp", bufs=4) as gp,
            tc.tile_pool(name="tp", bufs=2, space="PSUM") as tp,
            tc.tile_pool(name="mmp", bufs=1, space="PSUM") as mmp,
            tc.tile_pool(name="grup", bufs=1, space="PSUM") as grup,
        ):
            # ---- tiny setup (no DMA deps) ----
            dum = const.tile([1, 1], DT, tag="dum")
            nc.vector.memset(dum[:], 1.0)
            epsb = const.tile([P, 1], DT, tag="epsb")
            nc.vector.memset(epsb[:], 1e-36)
            dumob = const.tile([1, 1], BF, tag="dumob")
            # warm the ACT exp/ln table while DMAs are in flight
            nc.scalar.activation(dumob[:], dum[:], AF.Exp,
                                 bias=epsb[0:1, :])
            ident_bf = const.tile([P, P], BF, tag="ident_bf")
            make_identity(nc, ident_bf)
            ones_bf = const.tile([1, P], BF, tag="ones_bf")
            nc.vector.memset(ones_bf[:], 1.0)

            # ---- input DMAs: partition-halved across both HWDGE
            #      queues, strict need-order per queue; the tiny bias
            #      tensors ride the gpsimd SWDGE queue so they land
            #      early without stealing HWDGE bandwidth ----
            crit_s = const.tile([P, 384], BF, tag="crit_s")
            veT_s = const.tile([P, 512], BF, tag="veT_s")
            x1bf_s = const.tile([P, 256], BF, tag="x1bf_s")
            pB1_s = const.tile([P, 384], BF, tag="pB1_s")
            wihT_s = const.tile([P, 384], BF, tag="wihT_s")
            whhT_s = const.tile([P, 384], BF, tag="whhT_s")
            bcols_s = const.tile([P, 2], DT, tag="bcols_s")
            brow_s = const.tile([1, 512], BF, tag="brow_s")
            nc.gpsimd.dma_start(out=brow_s[:], in_=brow[:])
            nc.gpsimd.dma_start(out=bcols_s[:], in_=bcols[:])
            # crit gets the HBM to itself first (sync queue); scalar's
            # three triggers fire early so ACT is free by Exp time
            nc.sync.dma_start(out=crit_s[:], in_=crit[:])
            nc.scalar.dma_start(out=pB1_s[:], in_=pB1[:])
            nc.sync.dma_start(out=veT_s[:], in_=veTp[:])
            nc.scalar.dma_start(out=whhT_s[:], in_=whhTp[:])
            nc.scalar.dma_start(out=wihT_s[:], in_=wihTp[:])
            nc.sync.dma_start(out=x1bf_s[:], in_=x1bfp[:])

            x2T = crit_s[:, 0:256]
            M_wT = crit_s[:, 256:384]     # pre-scaled by T on host
            veT0 = veT_s[:, 0:256]
            veT1 = veT_s[:, 256:512]
            x1T = pB1_s[:, 0:256]
            W_wT = pB1_s[:, 256:384]
            wihT = wihT_s
            whhT = whhT_s
            x1bf = x1bf_s
            wbcol = bcols_s[:, 0:1]
            mbcol = bcols_s[:, 1:2]

            # ---- pmx[g,j] = T * (x2 @ M_w.T).T  (biasless) ----
            pmx = mmp.tile([P, N2], DT, tag="mm256", name="pmx")
            nc.tensor.matmul(pmx[:], lhsT=M_wT, rhs=x2T,
                             start=True, stop=True)
            negt = const.tile([P, 1], DT, tag="negt")
            nc.vector.tensor_reduce(out=negt[:], in_=pmx[:],
                                    axis=mybir.AxisListType.X, op=ALU.max,
                                    negate=True)
            # per j-half: exp -> PE transpose -> copy, pipelined
            ET = const.tile([P, N2], BF, tag="ET")
            E0 = const.tile([P, F], BF, tag="E0")
            E1 = const.tile([P, F], BF, tag="E1")
            for k, Ek in enumerate((E0, E1)):
                ks = slice(k * P, (k + 1) * P)
                nc.scalar.activation(ET[:, ks], pmx[:, ks], AF.Exp,
                                     bias=negt[:])
                pe = tp.tile([P, P], BF, tag="pe")
                nc.tensor.transpose(pe[:], ET[:, ks], ident_bf[:])
                nc.vector.tensor_copy(Ek[:], pe[:])

            # ---- S^T[g,i] = sum_j E[j,g] * veT[j,i]  (PE) ----
            pst = mmp.tile([P, N1], DT, tag="pst")
            nc.tensor.matmul(pst[:], lhsT=E0[:], rhs=veT0,
                             start=True, stop=False)
            nc.tensor.matmul(pst[:], lhsT=E1[:], rhs=veT1,
                             start=False, stop=True)
            # m1T (biasless) = (x1 @ W_w.T).T  -- fills the PE slot
            # between the S halves' dependencies
            pm1 = mmp.tile([P, N1], DT, tag="mm256", name="pm1")
            nc.tensor.matmul(pm1[:], lhsT=W_wT, rhs=x1T,
                             start=True, stop=True)

            # colmax2 = colmax + M_b = -negt/T + M_b  (for the m2 relu)
            colmax2 = const.tile([P, 1], DT, tag="colmax2")
            nc.vector.tensor_scalar(colmax2[:], negt[:], -1.0 / T, mbcol,
                                    op0=ALU.mult, op1=ALU.add)

            # ---- GRU pre-runs (no xT dependency): per tile PSUM bank
            #      PG = [gi_rz+gh_rz+b_rz (0:256) | gi_n+bih_n (256:384)
            #            | gh_n+bhh_n (384:512)] ----
            PGs = []
            for nt in range(2):
                ns = slice(nt * P, (nt + 1) * P)
                PG = grup.tile([P, 4 * F], DT, tag="PG", bufs=2,
                               name=f"PG{nt}")
                # bias row opens the group (zero-init + bias everywhere);
                # brow layout matches [b_rz | bih_n | bhh_n]
                nc.tensor.matmul(PG[:], lhsT=ones_bf[:], rhs=brow_s[:],
                                 start=True, stop=False)
                nc.tensor.matmul(PG[:, 0:2 * F], lhsT=x1T[:, ns],
                                 rhs=whhT[:, 0:2 * F], start=False, stop=False,
                                 skip_group_check=True)
                nc.tensor.matmul(PG[:, 3 * F:4 * F], lhsT=x1T[:, ns],
                                 rhs=whhT[:, 2 * F:3 * F], start=False, stop=False,
                                 skip_group_check=True)
                PGs.append(PG)

            # ---- LSE tail ----
            lnS = gp.tile([P, N1], DT, tag="lnS")
            nc.scalar.activation(lnS[:], pst[:], AF.Ln, bias=epsb[:])
            m2T = gp.tile([P, N1], DT, tag="m2T")
            nc.scalar.activation(m2T[:], lnS[:], AF.Relu,
                                 bias=colmax2[:], scale=1.0 / T)
            # warm the sigmoid/tanh table now -- ACT is done until the
            # GRU; the m2T read pins this after the Relu
            dumo2 = const.tile([1, 1], DT, tag="dumo2")
            nc.scalar.activation(dumo2[:], m2T[0:1, 0:1], AF.Sigmoid)
            # xs/xT on DVE so the table load overlaps them
            xs = gp.tile([P, N1], DT, tag="xs")
            nc.vector.tensor_add(xs[:], pm1[:], m2T[:])
            xT = const.tile([P, N1], BF, tag="xT")
            nc.vector.tensor_scalar(xT[:], xs[:], wbcol, 0.0,
                                    op0=ALU.add, op1=ALU.max)

            # ---- GRU post-xT: gi matmul, then the vector tail ----
            for nt in range(2):
                ns = slice(nt * P, (nt + 1) * P)
                x1_p = x1bf[:, ns]
                PG = PGs[nt]
                beng = nc.vector if nt == 0 else nc.gpsimd
                nc.tensor.matmul(PG[:, 0:F3], lhsT=xT[:, ns], rhs=wihT[:],
                                 start=False, stop=True,
                                 skip_group_check=True)
                rz = gp.tile([P, 2 * F], DT, tag="rz")
                nc.scalar.activation(rz[:], PG[:, 0:2 * F], AF.Sigmoid)
                rr, zz = rz[:, 0:F], rz[:, F:2 * F]
                # gpsimd can't read PSUM: t1/t2 stay on DVE
                t1 = gp.tile([P, F], DT, tag="t1")
                nc.vector.tensor_mul(t1[:], rr, PG[:, 3 * F:4 * F])
                t2 = gp.tile([P, F], DT, tag="t2")
                nc.vector.tensor_add(t2[:], t1[:], PG[:, 2 * F:F3])
                nn = gp.tile([P, F], DT, tag="nn")
                nc.scalar.activation(nn[:], t2[:], AF.Tanh)
                # omz/zx overlap the tanh; only t5/hh follow it
                omz = gp.tile([P, F], DT, tag="omz")
                beng.tensor_scalar(omz[:], zz, -1.0, 1.0,
                                   op0=ALU.mult, op1=ALU.add)
                zx = gp.tile([P, F], DT, tag="zx")
                beng.tensor_mul(zx[:], zz, x1_p)
                t5 = gp.tile([P, F], DT, tag="t5")
                nc.vector.tensor_mul(t5[:], omz[:], nn[:])
                hh = gp.tile([P, F], BF, tag=f"hh{nt}", name=f"hh{nt}")
                nc.vector.tensor_add(hh[:], t5[:], zx[:])
                # split each tile's output across both queues
                nc.scalar.dma_start(out=out[0:H, ns], in_=hh[0:H, :])
                nc.sync.dma_start(out=out[H:P, ns], in_=hh[H:P, :])

    # Walrus's TRN2 codegen allows at most one sync wait per instruction
    # (S3 LW struct). These Bacc passes split/move the extra waits.
    import bass_rust as _bass_rust
    _bass_rust.move_matmul_waits_to_ldweights(nc.m)
    bacc.Bacc.generate_event_semaphores(nc)
    bacc.Bacc.insert_library_loads(nc)
    mybir.codegen_inst_isa_subclasses(nc)
    return nc


_NC = None


def _in_maps(inputs):
    f32 = lambda a: np.ascontiguousarray(np.asarray(a), dtype=np.float32)
    bf = lambda a: np.ascontiguousarray(
        np.asarray(a, dtype=np.float32).astype(ml_dtypes.bfloat16))
    x1, x2, ve = (f32(inputs[k]) for k in ("x1", "x2", "valid_edge"))
    W_w, M_w = f32(inputs["W_w"]), f32(inputs["M_w"])
    W_b, M_b = f32(inputs["W_b"]), f32(inputs["M_b"])
    wih, whh = f32(inputs["gru_wih"]), f32(inputs["gru_whh"])
    bih, bhh = f32(inputs["gru_bih"]), f32(inputs["gru_bhh"])

    brow = np.empty((1, 512), np.float32)
    brow[0, 0:256] = bih[0:256] + bhh[0:256]
    brow[0, 256:384] = bih[256:384]
    brow[0, 384:512] = bhh[256:384]
    brow = bf(brow)

    M_wTb, W_wTb = bf(T * M_w.T), bf(W_w.T)
    wihTb, whhTb = bf(wih.T), bf(whh.T)
    bcols = np.empty((P, 2), np.float32)
    bcols[:, 0] = W_b
    bcols[:, 1] = M_b
    maps = []
    for b in range(B):
        veTb = bf(ve[b].T)
        crit = np.concatenate([bf(x2[b].T), M_wTb], axis=1)
        veTp = np.concatenate([veTb[0:P], veTb[P:2 * P]], axis=1)
        x1b = bf(x1[b])
        x1bfp = np.concatenate([x1b[0:P], x1b[P:2 * P]], axis=1)
        pB1 = np.concatenate([bf(x1[b].T), W_wTb], axis=1)
        maps.append({"crit": np.ascontiguousarray(crit),
                     "veTp": np.ascontiguousarray(veTp),
                     "x1bfp": np.ascontiguousarray(x1bfp),
                     "pB1": np.ascontiguousarray(pB1),
                     "wihTp": wihTb, "whhTp": whhTb,
                     "bcols": bcols, "brow": brow})
    return maps


def kernel(**inputs):
    global _NC
    if _NC is None:
        _NC = build()
    res = run_bass_kernel_spmd(_NC, _in_maps(inputs), list(range(B)))
    outs = []
    for b in range(B):
        o = np.asarray(res.results[b]["out"], dtype=np.float32)
        outs.append(np.concatenate([o[:, 0:F], o[:, F:2 * F]], axis=0))
    return np.stack(outs, axis=0).astype(np.float32)
